# revision 1
# baseline (speedup 1.0000x reference)
"""Causal self-attention Trainium2 Bass kernel (V4).

Full-input contract: kernel(**inputs) takes the unsharded inputs
(x [8,1024,768], W_attn [768,2304], b_attn [2304], W_proj [768,768],
b_proj [768]) and returns the full output [8,1024,768].

Sharding: data parallel - batch element b runs on NeuronCore b (B=8 =
n_cores), no collectives needed.

V4 changes vs V3 (trace-driven; V3 span 303us, PE 64% cov, ACT-bound
attention with a 6.5us single-partition reciprocal on the critical path):
  - scores land in bf16 PSUM tiles (1 bank instead of 2): per-head sp
    double-buffered + both heads' score MMs adjacent -> row-group
    concurrent (K=64 pairs at rows 0:64 / 64:128), single MM per tk-tile
    (N up to 1024 bf16 moving).
  - avp split into L/R [65,512] banks; L evicts at i=3 overlapping the
    R-half AV stream. PSUM: 4 sp banks + 4 avp banks = 8 exactly.
  - softmax divide: reciprocal_approx_fast straight off the PSUM l-row,
    1/l broadcast via gpsimd partition_broadcast (SBUF only), multiply
    fused into the avp eviction. No PE broadcast MMs, no yS staging copy.
  - bias adds: b_attn(v part) / b_proj partition-broadcast once via
    gpsimd; evictions become DVE tensor_tensor adds. Kills 32 K=1 bias
    MMs (the qk bias stays a per-partition tensor_scalar_add).
  - pb (exp scores) bf16: halves gpsimd affine_select + AV moving SBUF.
"""

import os
import sys

import numpy as np

for _p in ("/opt/trn_rl_repo", "/root/.axon_site/_ro/trn_rl_repo"):
    if os.path.isdir(_p) and _p not in sys.path:
        sys.path.insert(0, _p)
        break

import concourse.bass as bass
import concourse.mybir as mybir
import concourse.tile as tile
from concourse.bass_utils import run_bass_kernel_spmd
from concourse.masks import make_identity

T, C, H = 1024, 768, 12
C3 = 3 * C
NCORES = 8
NT = T // 128    # 8 t-tiles
NC_ = C // 128   # 6 c-tiles
NM = 2 * C // 128  # 12 m-tiles covering q,k output cols
f32 = mybir.dt.float32
f32r = mybir.dt.float32r
bf16 = mybir.dt.bfloat16

EXP = mybir.ActivationFunctionType.Exp


def build_module():
    md = f32r
    nc = bass.Bass()
    x_d = nc.dram_tensor("x", [T, C], md, kind="ExternalInput")
    wa_d = nc.dram_tensor("W_attn", [C, C3], md, kind="ExternalInput")
    ba_d = nc.dram_tensor("b_attn", [1, C3], md, kind="ExternalInput")
    wp_d = nc.dram_tensor("W_proj", [C, C], md, kind="ExternalInput")
    bp_d = nc.dram_tensor("b_proj", [1, C], md, kind="ExternalInput")
    out_d = nc.dram_tensor("out", [T, C], f32, kind="ExternalOutput")

    with tile.TileContext(nc) as tc:
        with tc.tile_pool(name="persist", bufs=1) as P0:
            identf = P0.tile([128, 128], f32, name="identf")
            make_identity(nc, identf[:])
            ident = P0.tile([128, 128], md, name="ident")
            nc.vector.tensor_copy(ident[:], identf[:])
            ones_f = P0.tile([128, 128], f32, name="ones_f")
            nc.vector.memset(ones_f[:], 1.0)
            ones_col = P0.tile([128, H], bf16, name="ones_col")
            nc.vector.tensor_copy(ones_col[:], ones_f[:, 0:H])

            qkT = [P0.tile([128, T], bf16, name=f"qkT{m}") for m in range(NM)]
            vA = [P0.tile([128, 65 * H], bf16, name=f"vA{t}") for t in range(NT)]
            yT = [P0.tile([128, T], md, name=f"yT{c}") for c in range(NC_)]
            ba_sb = P0.tile([1, C], md, name="ba_sb")
            bp_sb = P0.tile([1, C], md, name="bp_sb")
            baB = P0.tile([128, C], md, name="baB")   # b_attn v-part bcast
            bpB = P0.tile([128, C], md, name="bpB")   # b_proj bcast
            wpt = [P0.tile([128, C], md, name=f"wp{c}") for c in range(NC_)]
            bqk = [P0.tile([128, 1], f32, name=f"bqk{m}") for m in range(NM)]

            # preload the exp table set while ACT is idle (else the first
            # attention exp pays the ~2.7us ACT_TABLE_LOAD inline)
            warm = P0.tile([1, 16], f32, name="warm")
            nc.scalar.activation(warm[:], ones_f[0:1, 0:16], EXP, scale=0.125)

            # ---- phase 1: x load (FIRST DMAs issued) + transpose ----
            with tc.tile_pool(name="sb1", bufs=3) as SB1:
                xT = [SB1.tile([128, T], md, name=f"xT{c}", tag=f"xT{c}", bufs=1)
                      for c in range(NC_)]
                with tc.tile_pool(name="ps1", bufs=1, space="PSUM") as PS1:
                    for j2 in range(2):
                        trs = [PS1.tile([128, 512], md, tag=f"tr{c}", name=f"tr{c}")
                               for c in range(NC_)]
                        for u in range(4):
                            t = 4 * j2 + u
                            xt = SB1.tile([128, C], md, tag="xt", name="xt")
                            nc.sync.dma_start(out=xt[:], in_=x_d[128 * t:128 * (t + 1), :])
                            for c in range(NC_):
                                nc.tensor.transpose(trs[c][:, 128 * u:128 * (u + 1)],
                                                    xt[:, 128 * c:128 * (c + 1)], ident[:])
                        for c in range(NC_):
                            nc.vector.tensor_copy(xT[c][:, 512 * j2:512 * (j2 + 1)],
                                                  trs[c][:])

                # ---- phase 2: v then q^T/k^T ----
                with tc.tile_pool(name="ps12", bufs=2, space="PSUM") as PS12, \
                     tc.tile_pool(name="sb12", bufs=3) as SB12:
                    # v: stationary xT tiles, moving W_v columns
                    wV = [SB12.tile([128, C], md, name=f"wV{c}", tag=f"wV{c}", bufs=1)
                          for c in range(NC_)]
                    for c in range(NC_):
                        nc.sync.dma_start(out=wV[c][:],
                                          in_=wa_d[128 * c:128 * (c + 1), 2 * C:3 * C])
                    nc.sync.dma_start(out=ba_sb[:], in_=ba_d[0:1, 2 * C:3 * C])
                    # one-time bias broadcast (free-dim stride-0 DMA
                    # replicate); scalar-queue-issued: ACT is idle here and
                    # this keeps the sync queue free for weight loads
                    nc.scalar.dma_start(
                        out=baB[:],
                        in_=ba_sb[0:1, :].unsqueeze(1).to_broadcast([1, 128, C]))
                    baB_r = baB.rearrange("p (h e) -> p h e", h=H)
                    for t in range(NT):
                        accv = PS12.tile([128, C], f32, tag="v", name="accv")
                        for c in range(NC_):
                            xcol = xT[c][:, 128 * t:128 * (t + 1)]
                            nc.tensor.matmul(accv[:, 0:512], xcol, wV[c][:, 0:512],
                                             start=(c == 0), stop=(c == NC_ - 1))
                            nc.tensor.matmul(accv[:, 512:C], xcol, wV[c][:, 512:C],
                                             start=(c == 0), stop=(c == NC_ - 1))
                        av = vA[t].rearrange("p (h e) -> p h e", h=H)
                        nc.vector.tensor_copy(
                            av[:, :, 64:65],
                            ones_col.rearrange("p (h o) -> p h o", o=1))
                        # eviction with fused bias add (replaces bias MMs)
                        nc.vector.tensor_tensor(
                            av[:, :, 0:64],
                            accv[:].rearrange("p (h e) -> p h e", h=H),
                            baB_r[:, :, 0:64],
                            mybir.AluOpType.add)

                    # q^T / k^T: W_attn halves batched per c-tile; the k half
                    # reuses the wV slots (freed once the v matmuls finish).
                    # bqk partition-scatter DMAs (4B-granular, slow to issue)
                    # go on the scalar HWDGE queue, off the weight-load path
                    for m in range(NM):
                        nc.scalar.dma_start(
                            out=bqk[m][:],
                            in_=ba_d.bitcast(f32)[0:1, 128 * m:128 * (m + 1)]
                                .rearrange("a p -> p a"))
                    wAq = [SB12.tile([128, C], md, name=f"wAq{c}", tag=f"wAq{c}",
                                     bufs=1) for c in range(NC_)]
                    for half in range(2):
                        if half == 0:
                            wh = wAq
                        else:
                            wh = [SB12.tile([128, C], md, name=f"wAk{c}",
                                            tag=f"wV{c}", bufs=1)
                                  for c in range(NC_)]
                        for c in range(NC_):
                            nc.sync.dma_start(
                                out=wh[c][:],
                                in_=wa_d[128 * c:128 * (c + 1), C * half:C * (half + 1)])
                        for mm in range(NC_):
                            m = NC_ * half + mm
                            acc = PS12.tile([128, T], f32, tag="qk", name="acc")
                            for c in range(NC_):
                                wa = wh[c][:, 128 * mm:128 * (mm + 1)]
                                for j2 in range(2):
                                    nc.tensor.matmul(
                                        acc[:, 512 * j2:512 * (j2 + 1)],
                                        wa,
                                        xT[c][:, 512 * j2:512 * (j2 + 1)],
                                        start=(c == 0), stop=(c == NC_ - 1),
                                    )
                            # psum -> sbuf(bf16) with per-partition bias add
                            nc.vector.tensor_scalar_add(qkT[m][:], acc[:], bqk[m][:])

            # ---- phase 3: attention (head pairs; pipelined scores->AV) ----
            # W_proj / b_proj loads issued here: sync queue is idle now and
            # phase 4 needs them much later
            nc.sync.dma_start(out=bp_sb[:], in_=bp_d[:])
            for c in range(NC_):
                nc.sync.dma_start(out=wpt[c][:], in_=wp_d[128 * c:128 * (c + 1), :])
            nc.scalar.dma_start(
                out=bpB[:],
                in_=bp_sb[0:1, :].unsqueeze(1).to_broadcast([1, 128, C]))
            with tc.tile_pool(name="ps3", bufs=1, space="PSUM") as PS3, \
                 tc.tile_pool(name="sb3", bufs=4) as SB3:
                from collections import deque
                pending = deque()   # deferred normalization pipeline stages

                for hp in range(H // 2):
                    qt = qkT[hp]
                    kt = qkT[NC_ + hp]

                    # per-head-pair state
                    sps = {}        # (i, hs) -> score tile (live window)
                    avs = {}        # (hs, half) -> [65,512] accumulator
                    # (allocated lazily in emit_av so the alloc lands after
                    # the previous pair's staging copy frees the bank)

                    def emit_score(i):
                        lo = 128 * i
                        for hs in range(2):
                            base = 64 * hs
                            sp = PS3.tile([128, T], f32, tag="s", bufs=2,
                                          name=f"sp{hs}")
                            ktile = kt[base:base + 64, lo:lo + 128]
                            if lo < 512:
                                nc.tensor.matmul(sp[:, lo:512], ktile,
                                                 qt[base:base + 64, lo:512],
                                                 start=True, stop=True)
                                nc.tensor.matmul(sp[:, 512:T], ktile,
                                                 qt[base:base + 64, 512:T],
                                                 start=True, stop=True)
                            else:
                                nc.tensor.matmul(sp[:, lo:T], ktile,
                                                 qt[base:base + 64, lo:T],
                                                 start=True, stop=True)
                            sps[(i, hs)] = sp

                    def emit_exp_sel(i):
                        lo = 128 * i
                        for hs in range(2):
                            sp = sps.pop((i, hs))
                            pb = SB3.tile([128, T], bf16, tag="pb", bufs=6,
                                          name="pb")
                            nc.scalar.activation(pb[:, lo:T], sp[:, lo:T], EXP,
                                                 scale=0.125)
                            # diagonal [128,128] sub-tile: keep iff p <= f
                            nc.gpsimd.affine_select(
                                out=pb[:, lo:lo + 128], in_=pb[:, lo:lo + 128],
                                pattern=[[1, 128]],
                                compare_op=mybir.AluOpType.is_ge, fill=0.0,
                                base=0, channel_multiplier=-1,
                            )
                            sps[("pb", i, hs)] = pb

                    def emit_av(i):
                        lo = 128 * i
                        for hs in range(2):
                            h = 2 * hp + hs
                            pb = sps.pop(("pb", i, hs))
                            vt = vA[i][:, 65 * h:65 * h + 65]
                            if i == 0:
                                for half in range(2):
                                    avs[(hs, half)] = PS3.tile(
                                        [65, 512], f32, tag=f"av{hs}{half}",
                                        bufs=1, name=f"av{hs}{half}")
                            if lo < 512:
                                nc.tensor.matmul(avs[(hs, 0)][0:65, lo:512], vt,
                                                 pb[:, lo:512], start=(i == 0),
                                                 stop=(i == 3),
                                                 skip_group_check=True)
                                nc.tensor.matmul(avs[(hs, 1)][0:65, 0:512], vt,
                                                 pb[:, 512:T], start=(i == 0),
                                                 stop=(i == NT - 1),
                                                 skip_group_check=True)
                            else:
                                nc.tensor.matmul(avs[(hs, 1)][0:65, lo - 512:512],
                                                 vt, pb[:, lo:T], start=False,
                                                 stop=(i == NT - 1),
                                                 skip_group_check=True)

                    def norm_stages(myhp, half, myavs):
                        # Normalization as five deferred pipeline stages.
                        # Emitted spread across later iterations so every
                        # stage's input is long-done when its (in-order)
                        # engine queue reaches it - a blocked stage at a
                        # queue head stalls everything behind it, which was
                        # worth ~15us per head-pair when emitted inline.
                        st = {}

                        def s0():   # PSUM -> SBUF staging (frees the banks)
                            for hs in range(2):
                                yU = SB3.tile([65, 512], f32, tag=f"yU{hs}",
                                              bufs=3, name=f"yU{hs}")
                                nc.vector.tensor_copy(
                                    yU[:], myavs[(hs, half)][0:65, 0:512])
                                st[hs] = yU

                        def s1():   # gather the two l rows (ACT; tiny)
                            lp = SB3.tile([33, 512], f32, tag="lp", bufs=2,
                                          name="lp")
                            nc.scalar.copy(lp[0:1, :], st[0][64:65, :])
                            nc.scalar.copy(lp[32:33, :], st[1][64:65, :])
                            st["lp"] = lp

                        def s2():   # batched reciprocal (rows 0 and 32)
                            rl = SB3.tile([33, 512], f32, tag="rl", bufs=2,
                                          name="rl")
                            nc.vector.reciprocal(rl[:], st["lp"][:])
                            st["rl"] = rl

                        def s3():   # replicate 1/l across partitions (DMA)
                            for hs in range(2):
                                rlb = SB3.tile([64, 512], f32, tag="rlb",
                                               bufs=4, name="rlb")
                                nc.sync.dma_start(
                                    out=rlb[:],
                                    in_=st["rl"][32 * hs:32 * hs + 1, :]
                                        .unsqueeze(1).to_broadcast([1, 64, 512]))
                                st[("rlb", hs)] = rlb

                        def s4():   # normalize into yT
                            for hs in range(2):
                                base = 64 * hs
                                nc.vector.tensor_tensor(
                                    yT[myhp][base:base + 64,
                                             512 * half:512 * (half + 1)],
                                    st[hs][0:64, :], st[("rlb", hs)][:],
                                    mybir.AluOpType.mult)

                        return [s0, s1, s2, s3, None, s4]

                    def pop_pending(k=2):
                        n = 0
                        while pending and n < k:
                            s = pending.popleft()
                            if s is not None:
                                s()
                            n += 1

                    # software-pipelined emission: scores one tk-tile ahead
                    # of the AV stream; deferred norm stages drain two per
                    # iteration in the gaps
                    pop_pending()
                    emit_score(0)
                    emit_exp_sel(0)
                    for i in range(1, NT):
                        pop_pending()
                        emit_score(i)
                        emit_exp_sel(i)
                        emit_av(i - 1)
                        if i == 5:
                            pending.extend(norm_stages(hp, 0, avs))
                    emit_av(NT - 1)
                    pending.extend(norm_stages(hp, 1, avs))

                # drain the tail (last pair's normalization)
                while pending:
                    s = pending.popleft()
                    if s is not None:
                        s()

            # ---- phase 4: out = y^T.T @ W_proj + b_proj ----
            with tc.tile_pool(name="ps4", bufs=2, space="PSUM") as PS4, \
                 tc.tile_pool(name="sb4", bufs=3) as SB4:
                for t in range(NT):
                    acc = PS4.tile([128, C], f32, tag="pj", name="acc")
                    for c in range(NC_):
                        ycol = yT[c][:, 128 * t:128 * (t + 1)]
                        nc.tensor.matmul(acc[:, 0:512], ycol, wpt[c][:, 0:512],
                                         start=(c == 0), stop=(c == NC_ - 1))
                        nc.tensor.matmul(acc[:, 512:C], ycol, wpt[c][:, 512:C],
                                         start=(c == 0), stop=(c == NC_ - 1))
                    ot = SB4.tile([128, C], f32, tag="ot", bufs=3, name="ot")
                    # eviction with fused bias add (replaces bias MMs)
                    nc.vector.tensor_tensor(ot[:], acc[:], bpB[:],
                                            mybir.AluOpType.add)
                    nc.sync.dma_start(out=out_d[128 * t:128 * (t + 1), :], in_=ot[:])

    return nc


_WAIT_SKIP = {"InstNoOp", "InstEventSemOp", "InstSemaphoreOp",
              "InstCustomDveAnt", "InstPartitionBroadcast",
              "InstPartitionAllReduce"}


def _legalize_waits(nc):
    """walrus's codegen allows limited sync-wait commands per ISA struct
    (e.g. a Matmult's waits all land on the generated LDWEIGHTS struct which
    has one slot). Move excess waits onto same-engine NoOps inserted
    immediately before the instruction - program order on the engine queue
    preserves the synchronization semantics."""
    nfix = 0
    for fn in nc.m.functions:
        for bb in fn.blocks:
            out = []
            for ins in bb.instructions:
                si = ins.sync_info
                if (type(ins).__name__ not in _WAIT_SKIP and si is not None
                        and si.on_wait and len(si.on_wait) > 1):
                    waits = list(si.on_wait)
                    extra, keep = waits[:-1], waits[-1:]
                    for k, w in enumerate(extra):
                        nop = mybir.InstNoOp(name=f"{ins.name}-wf{k}", ins=[], outs=[])
                        nop.engine = ins.engine
                        nop.sync_info = mybir.SyncInfo(on_wait=[w], on_update=[])
                        out.append(nop)
                    ins.sync_info = mybir.SyncInfo(
                        on_wait=keep, on_update=list(si.on_update or []))
                    nfix += 1
                out.append(ins)
            bb.instructions = out
    return nfix


_cached_module = None


def _get_module():
    global _cached_module
    if _cached_module is None:
        nc = build_module()
        _legalize_waits(nc)
        _cached_module = nc
    return _cached_module


def make_in_maps(x, W_attn, b_attn, W_proj, b_proj):
    x = np.asarray(x, dtype=np.float32)
    wa = np.ascontiguousarray(np.asarray(W_attn, dtype=np.float32))
    ba = np.ascontiguousarray(np.asarray(b_attn, dtype=np.float32).reshape(1, C3))
    wp = np.ascontiguousarray(np.asarray(W_proj, dtype=np.float32))
    bp = np.ascontiguousarray(np.asarray(b_proj, dtype=np.float32).reshape(1, C))
    return [
        dict(x=np.ascontiguousarray(x[b]), W_attn=wa, b_attn=ba, W_proj=wp, b_proj=bp)
        for b in range(x.shape[0])
    ]


def run(x, W_attn, b_attn, W_proj, b_proj, trace=False, **spmd_kwargs):
    nc = _get_module()
    in_maps = make_in_maps(x, W_attn, b_attn, W_proj, b_proj)
    res = run_bass_kernel_spmd(nc, in_maps, list(range(NCORES)), trace=trace,
                               **spmd_kwargs)
    out = np.stack([res.results[b]["out"] for b in range(len(in_maps))], axis=0)
    return out, res


def kernel(x, W_attn, b_attn, W_proj, b_proj):
    out, _ = run(x, W_attn, b_attn, W_proj, b_proj)
    return out



# revision 4
# speedup vs baseline: 1.0514x; 1.0514x over previous
"""Causal self-attention Trainium2 Bass kernel (V5).

Full-input contract: kernel(**inputs) takes the unsharded inputs
(x [8,1024,768], W_attn [768,2304], b_attn [2304], W_proj [768,768],
b_proj [768]) and returns the full output [8,1024,768].

Sharding: data parallel - batch element b runs on NeuronCore b (B=8 =
n_cores), no collectives needed.

V5 changes vs V4 (trace-driven; V4 331us, PE 60% cov, HAM throttled to
K=4/8 for the entire 188us attention phase, 40us DVE reciprocal, 88us
ACT exp):
  - host-side prep: x is transposed and cast to bf16 on the host
    (xT input [768,1024]); W_attn split into Wq/Wk/Wv and cast bf16;
    W_proj bf16. Kills the 48 PE transposes + DVE copies of phase 1 and
    halves weight DMA bytes.
  - all GEMMs run on bf16 operands (fp32 PSUM accumulation).
  - attention pipeline unit is a (k-tile, col-half) CHUNK with fp32
    scores in a [128, 2x512] PSUM tile (2 banks, both heads of the
    pair). Chunks are double-buffered (4 banks) next to the 4 avp
    banks, so the score MM for chunk n+2 no longer waits on exp(n):
    the PE never idles long enough for HAM to re-throttle.
  - one exp per chunk covers both heads ([128, 2, n] AP) - halves ACT
    instruction count; one affine_select masks both heads' diagonal.
  - softmax divide: l-rows gathered by SBUF->SBUF DMA into a [4,512]
    tile per head pair, ONE reciprocal_approx_fast (the V4 kernel spent
    40us in 12 full-precision Newton reciprocals), DMA broadcast,
    DVE multiply fused into yT (bf16).
"""

import os
import sys

import numpy as np

for _p in ("/opt/trn_rl_repo", "/root/.axon_site/_ro/trn_rl_repo"):
    if os.path.isdir(_p) and _p not in sys.path:
        sys.path.insert(0, _p)
        break

import concourse.bass as bass
import concourse.mybir as mybir
import concourse.tile as tile
from concourse.bass_utils import run_bass_kernel_spmd

T, C, H = 1024, 768, 12
C3 = 3 * C
NCORES = 8
NT = T // 128    # 8 t-tiles
NC_ = C // 128   # 6 c-tiles
NHP = H // 2     # 6 head pairs
f32 = mybir.dt.float32
bf16 = mybir.dt.bfloat16

EXP = mybir.ActivationFunctionType.Exp


def build_module():
    nc = bass.Bass()
    xT_d = nc.dram_tensor("xT", [C, T], bf16, kind="ExternalInput")
    wq_d = nc.dram_tensor("Wq", [C, C], bf16, kind="ExternalInput")
    wk_d = nc.dram_tensor("Wk", [C, C], bf16, kind="ExternalInput")
    wv_d = nc.dram_tensor("Wv", [C, C], bf16, kind="ExternalInput")
    wp_d = nc.dram_tensor("Wp", [C, C], bf16, kind="ExternalInput")
    ba_d = nc.dram_tensor("b_attn", [1, C3], f32, kind="ExternalInput")
    bp_d = nc.dram_tensor("b_proj", [1, C], f32, kind="ExternalInput")
    out_d = nc.dram_tensor("out", [T, C], f32, kind="ExternalOutput")

    with tile.TileContext(nc) as tc:
        with tc.tile_pool(name="persist", bufs=1) as P0:
            qkT = [P0.tile([128, T], bf16, name=f"qkT{m}") for m in range(2 * NC_)]
            vA = [P0.tile([128, 65 * H], bf16, name=f"vA{t}") for t in range(NT)]
            yT = [P0.tile([128, T], bf16, name=f"yT{c}") for c in range(NC_)]
            ba_sb = P0.tile([1, C], f32, name="ba_sb")
            bp_sb = P0.tile([1, C], f32, name="bp_sb")
            baB = P0.tile([128, C], f32, name="baB")   # b_attn v-part bcast
            bpB = P0.tile([128, C], f32, name="bpB")   # b_proj bcast
            wpt = [P0.tile([128, C], bf16, name=f"wp{c}") for c in range(NC_)]
            bqk = [P0.tile([128, 1], f32, name=f"bqk{m}") for m in range(2 * NC_)]
            ones_col = P0.tile([128, H], bf16, name="ones_col")
            nc.vector.memset(ones_col[:], 1.0)
            warm_src = P0.tile([1, 16], f32, name="warm_src")
            nc.vector.memset(warm_src[:], 1.0)

            # preload the exp table while ACT is idle (else the first
            # attention exp pays the ~2.7us ACT_TABLE_LOAD inline)
            warm = P0.tile([1, 16], f32, name="warm")
            nc.scalar.activation(warm[:], warm_src[:], EXP, scale=0.125)

            # ---- phase A: qkv GEMMs (x arrives pre-transposed bf16) ----
            with tc.tile_pool(name="sbA", bufs=1) as SBA:
                xT = [SBA.tile([128, T], bf16, name=f"xT{c}", tag=f"xT{c}",
                               bufs=1) for c in range(NC_)]
                for c in range(NC_):
                    nc.sync.dma_start(out=xT[c][:],
                                      in_=xT_d[128 * c:128 * (c + 1), :])
                wV = [SBA.tile([128, C], bf16, name=f"wV{c}", tag=f"wV{c}",
                               bufs=1) for c in range(NC_)]
                for c in range(NC_):
                    nc.sync.dma_start(out=wV[c][:],
                                      in_=wv_d[128 * c:128 * (c + 1), :])
                nc.sync.dma_start(out=ba_sb[:], in_=ba_d[0:1, 2 * C:3 * C])
                # one-time bias broadcast (free-dim stride-0 DMA replicate);
                # scalar-queue-issued to keep the sync queue free for weights
                nc.scalar.dma_start(
                    out=baB[:],
                    in_=ba_sb[0:1, :].unsqueeze(1).to_broadcast([1, 128, C]))
                baB_r = baB.rearrange("p (h e) -> p h e", h=H)
                # bqk partition-scatter DMAs (4B-granular, slow to issue) on
                # the scalar HWDGE queue, off the weight-load path
                for m in range(2 * NC_):
                    nc.scalar.dma_start(
                        out=bqk[m][:],
                        in_=ba_d[0:1, 128 * m:128 * (m + 1)]
                            .rearrange("a p -> p a"))
                # q/k weight loads stream behind the v weights
                wQ = [SBA.tile([128, C], bf16, name=f"wQ{c}", tag=f"wQ{c}",
                               bufs=1) for c in range(NC_)]
                wK = [SBA.tile([128, C], bf16, name=f"wK{c}", tag=f"wK{c}",
                               bufs=1) for c in range(NC_)]
                for c in range(NC_):
                    nc.sync.dma_start(out=wQ[c][:],
                                      in_=wq_d[128 * c:128 * (c + 1), :])
                for c in range(NC_):
                    nc.sync.dma_start(out=wK[c][:],
                                      in_=wk_d[128 * c:128 * (c + 1), :])

                with tc.tile_pool(name="psA", bufs=1, space="PSUM") as PSA:
                    # v: stationary xT columns, moving W_v rows
                    for t in range(NT):
                        accv = PSA.tile([128, C], f32, tag="v", bufs=2,
                                        name="accv")
                        for c in range(NC_):
                            xcol = xT[c][:, 128 * t:128 * (t + 1)]
                            nc.tensor.matmul(accv[:, 0:512], xcol,
                                             wV[c][:, 0:512],
                                             start=(c == 0), stop=(c == NC_ - 1))
                            nc.tensor.matmul(accv[:, 512:C], xcol,
                                             wV[c][:, 512:C],
                                             start=(c == 0), stop=(c == NC_ - 1))
                        av = vA[t].rearrange("p (h e) -> p h e", h=H)
                        nc.vector.tensor_copy(
                            av[:, :, 64:65],
                            ones_col.rearrange("p (h o) -> p h o", o=1))
                        # eviction with fused bias add
                        nc.vector.tensor_tensor(
                            av[:, :, 0:64],
                            accv[:].rearrange("p (h e) -> p h e", h=H),
                            baB_r[:, :, 0:64],
                            mybir.AluOpType.add)

                    # q^T / k^T: out rows = W columns (m-tile), moving xT
                    for half in range(2):
                        wh = wQ if half == 0 else wK
                        for mm in range(NC_):
                            m = NC_ * half + mm
                            acc = PSA.tile([128, T], f32, tag="qk", bufs=2,
                                           name="acc")
                            for c in range(NC_):
                                wa = wh[c][:, 128 * mm:128 * (mm + 1)]
                                for j2 in range(2):
                                    nc.tensor.matmul(
                                        acc[:, 512 * j2:512 * (j2 + 1)],
                                        wa,
                                        xT[c][:, 512 * j2:512 * (j2 + 1)],
                                        start=(c == 0), stop=(c == NC_ - 1),
                                    )
                            # psum -> sbuf(bf16) with per-partition bias add
                            nc.vector.tensor_scalar_add(qkT[m][:], acc[:],
                                                        bqk[m][:])

            # ---- phase B: attention ----
            # W_proj / b_proj loads issued here: sync queue is idle now and
            # phase C needs them much later
            nc.sync.dma_start(out=bp_sb[:], in_=bp_d[:])
            for c in range(NC_):
                nc.sync.dma_start(out=wpt[c][:],
                                  in_=wp_d[128 * c:128 * (c + 1), :])
            nc.scalar.dma_start(
                out=bpB[:],
                in_=bp_sb[0:1, :].unsqueeze(1).to_broadcast([1, 128, C]))

            with tc.tile_pool(name="ps3", bufs=1, space="PSUM") as PS3, \
                 tc.tile_pool(name="sb3", bufs=1) as SB3:
                from collections import deque
                pending = deque()   # deferred normalization pipeline stages

                def pop_pending(k=2):
                    n = 0
                    while pending and n < k:
                        s = pending.popleft()
                        if s is not None:
                            s()
                        n += 1

                # chunk list: (i, w); w=0 -> query cols [lo,512) (i<4 only),
                # w=1 -> [max(lo,512), 1024)
                chunks = []
                for i in range(NT):
                    if i < 4:
                        chunks.append((i, 0))
                        chunks.append((i, 1))
                    else:
                        chunks.append((i, 1))

                def chunk_cols(i, w):
                    lo = 128 * i
                    if w == 0:
                        return lo, 512
                    return max(lo, 512), T

                for hp in range(NHP):
                    qt = qkT[hp]
                    kt = qkT[NC_ + hp]
                    sps = {}   # chunk -> score PSUM tile
                    pbs = {}   # chunk -> exp'd SBUF tile
                    avs = {}   # (hs, half) -> [65,512] accumulator
                    st = {}    # (hs, half) -> staged yU tile

                    def emit_score(ch):
                        i, w = ch
                        lo = 128 * i
                        c0, c1 = chunk_cols(i, w)
                        scp = PS3.tile([128, 1024], f32, tag="sc", bufs=2,
                                       name="scp")
                        for hs in range(2):
                            base = 64 * hs
                            nc.tensor.matmul(
                                scp[:, 512 * hs:512 * hs + (c1 - c0)],
                                kt[base:base + 64, lo:lo + 128],
                                qt[base:base + 64, c0:c1],
                                start=True, stop=True)
                        sps[ch] = scp

                    def emit_exp(ch):
                        i, w = ch
                        lo = 128 * i
                        c0, c1 = chunk_cols(i, w)
                        n = c1 - c0
                        scp = sps.pop(ch)
                        pb = SB3.tile([128, 1024], bf16, tag="pb", bufs=4,
                                      name="pb")
                        scv = scp.rearrange("p (s n) -> p s n", s=2)
                        pbv = pb.rearrange("p (s n) -> p s n", s=2)
                        nc.scalar.activation(pbv[:, :, 0:n], scv[:, :, 0:n],
                                             EXP, scale=0.125)
                        if c0 == lo:
                            # diagonal [128,128] block (both heads):
                            # keep iff q - key >= 0
                            nc.gpsimd.affine_select(
                                out=pbv[:, :, 0:128], in_=pbv[:, :, 0:128],
                                pattern=[[0, 2], [1, 128]],
                                compare_op=mybir.AluOpType.is_ge, fill=0.0,
                                base=0, channel_multiplier=-1,
                            )
                        pbs[ch] = pb

                    def emit_av(ch):
                        i, w = ch
                        c0, c1 = chunk_cols(i, w)
                        n = c1 - c0
                        pb = pbs.pop(ch)
                        if i == 0 and w == 0:
                            for hs in range(2):
                                for half in range(2):
                                    avs[(hs, half)] = PS3.tile(
                                        [65, 512], f32, tag=f"av{hs}{half}",
                                        bufs=1, name=f"av{hs}{half}")
                        for hs in range(2):
                            h = 2 * hp + hs
                            vt = vA[i][:, 65 * h:65 * h + 65]
                            if w == 0:
                                nc.tensor.matmul(
                                    avs[(hs, 0)][0:65, c0:512], vt,
                                    pb[:, 512 * hs:512 * hs + n],
                                    start=(i == 0), stop=(i == 3),
                                    skip_group_check=True)
                            else:
                                nc.tensor.matmul(
                                    avs[(hs, 1)][0:65, c0 - 512:512], vt,
                                    pb[:, 512 * hs:512 * hs + n],
                                    start=(i == 0), stop=(i == NT - 1),
                                    skip_group_check=True)

                    def make_s0(myst, myavs, half):
                        def s0():   # PSUM -> SBUF staging (frees the banks)
                            for hs in range(2):
                                yU = SB3.tile([65, 512], f32,
                                              tag=f"yU{hs}{half}", bufs=2,
                                              name=f"yU{hs}{half}")
                                nc.vector.tensor_copy(
                                    yU[:], myavs[(hs, half)][0:65, 0:512])
                                myst[(hs, half)] = yU
                        return s0

                    def make_norm(myhp, myst):
                        # l-row gather (DMA), one fast reciprocal, DMA
                        # broadcast, DVE multiply into yT
                        loc = {}

                        def s_gather():
                            lr = SB3.tile([4, 512], f32, tag="lr", bufs=2,
                                          name="lr")
                            j = 0
                            for hs in range(2):
                                for half in range(2):
                                    nc.sync.dma_start(
                                        out=lr[j:j + 1, :],
                                        in_=myst[(hs, half)][64:65, :])
                                    j += 1
                            loc["lr"] = lr

                        def s_recip():
                            rl = SB3.tile([4, 512], f32, tag="rl", bufs=2,
                                          name="rl")
                            nc.vector.reciprocal_approx_fast(rl[:],
                                                             loc["lr"][:])
                            loc["rl"] = rl

                        def s_bcast():
                            j = 0
                            for hs in range(2):
                                for half in range(2):
                                    rlb = SB3.tile([64, 512], f32,
                                                   tag=f"rlb{j}", bufs=2,
                                                   name=f"rlb{j}")
                                    nc.scalar.dma_start(
                                        out=rlb[:],
                                        in_=loc["rl"][j:j + 1, :].unsqueeze(1)
                                            .to_broadcast([1, 64, 512]))
                                    loc[("rlb", hs, half)] = rlb
                                    j += 1

                        def s_mult():
                            for hs in range(2):
                                base = 64 * hs
                                for half in range(2):
                                    nc.vector.tensor_tensor(
                                        yT[myhp][base:base + 64,
                                                 512 * half:512 * (half + 1)],
                                        myst[(hs, half)][0:64, :],
                                        loc[("rlb", hs, half)][:],
                                        mybir.AluOpType.mult)

                        return [s_gather, None, s_recip, s_bcast, None,
                                s_mult]

                    # software-pipelined emission: scores one chunk ahead of
                    # the AV stream; deferred norm stages drain in the gaps
                    pop_pending()
                    emit_score(chunks[0])
                    emit_exp(chunks[0])
                    for n_ in range(1, len(chunks)):
                        pop_pending()
                        emit_score(chunks[n_])
                        emit_exp(chunks[n_])
                        emit_av(chunks[n_ - 1])
                        if chunks[n_ - 1] == (3, 0):
                            pending.append(make_s0(st, avs, 0))
                    emit_av(chunks[-1])
                    pending.append(make_s0(st, avs, 1))
                    pending.extend(make_norm(hp, st))

                # drain the tail (last pair's normalization)
                while pending:
                    s = pending.popleft()
                    if s is not None:
                        s()

            # ---- phase C: out = y^T.T @ W_proj + b_proj ----
            with tc.tile_pool(name="ps4", bufs=2, space="PSUM") as PS4, \
                 tc.tile_pool(name="sb4", bufs=3) as SB4:
                for t in range(NT):
                    acc = PS4.tile([128, C], f32, tag="pj", name="acc")
                    for c in range(NC_):
                        ycol = yT[c][:, 128 * t:128 * (t + 1)]
                        nc.tensor.matmul(acc[:, 0:512], ycol,
                                         wpt[c][:, 0:512],
                                         start=(c == 0), stop=(c == NC_ - 1))
                        nc.tensor.matmul(acc[:, 512:C], ycol,
                                         wpt[c][:, 512:C],
                                         start=(c == 0), stop=(c == NC_ - 1))
                    ot = SB4.tile([128, C], f32, tag="ot", bufs=3, name="ot")
                    nc.vector.tensor_tensor(ot[:], acc[:], bpB[:],
                                            mybir.AluOpType.add)
                    nc.sync.dma_start(out=out_d[128 * t:128 * (t + 1), :],
                                      in_=ot[:])

    return nc


_WAIT_SKIP = {"InstNoOp", "InstEventSemOp", "InstSemaphoreOp",
              "InstPartitionBroadcast", "InstPartitionAllReduce"}


def _legalize_waits(nc):
    """walrus's codegen allows limited sync-wait commands per ISA struct
    (e.g. a Matmult's waits all land on the generated LDWEIGHTS struct which
    has one slot). Move excess waits onto same-engine NoOps inserted
    immediately before the instruction - program order on the engine queue
    preserves the synchronization semantics."""
    nfix = 0
    for fn in nc.m.functions:
        for bb in fn.blocks:
            out = []
            for ins in bb.instructions:
                si = ins.sync_info
                if (type(ins).__name__ not in _WAIT_SKIP and si is not None
                        and si.on_wait and len(si.on_wait) > 1):
                    waits = list(si.on_wait)
                    extra, keep = waits[:-1], waits[-1:]
                    for k, w in enumerate(extra):
                        nop = mybir.InstNoOp(name=f"{ins.name}-wf{k}", ins=[],
                                             outs=[])
                        nop.engine = ins.engine
                        nop.sync_info = mybir.SyncInfo(on_wait=[w],
                                                       on_update=[])
                        out.append(nop)
                    ins.sync_info = mybir.SyncInfo(
                        on_wait=keep, on_update=list(si.on_update or []))
                    nfix += 1
                out.append(ins)
            bb.instructions = out
    return nfix


_cached_module = None


def _get_module():
    global _cached_module
    if _cached_module is None:
        nc = build_module()
        # populate .instr bytes for InstCustomDveAnt (reciprocal_approx_fast)
        # - Bacc.compile() runs this pass but the raw-Bass path doesn't, and
        # walrus codegen fails with "ISA wrong length" on empty .instr
        mybir.codegen_inst_isa_subclasses(nc)
        _legalize_waits(nc)
        _cached_module = nc
    return _cached_module


def make_in_maps(x, W_attn, b_attn, W_proj, b_proj):
    import ml_dtypes
    bf = ml_dtypes.bfloat16
    x = np.asarray(x, dtype=np.float32)
    wa = np.asarray(W_attn, dtype=np.float32)
    wq = np.ascontiguousarray(wa[:, 0:C].astype(bf))
    wk = np.ascontiguousarray(wa[:, C:2 * C].astype(bf))
    wv = np.ascontiguousarray(wa[:, 2 * C:3 * C].astype(bf))
    wp = np.ascontiguousarray(np.asarray(W_proj, dtype=np.float32).astype(bf))
    ba = np.ascontiguousarray(
        np.asarray(b_attn, dtype=np.float32).reshape(1, C3))
    bp = np.ascontiguousarray(
        np.asarray(b_proj, dtype=np.float32).reshape(1, C))
    return [
        dict(xT=np.ascontiguousarray(x[b].T.astype(bf)),
             Wq=wq, Wk=wk, Wv=wv, Wp=wp, b_attn=ba, b_proj=bp)
        for b in range(x.shape[0])
    ]


def run(x, W_attn, b_attn, W_proj, b_proj, trace=False, **spmd_kwargs):
    nc = _get_module()
    in_maps = make_in_maps(x, W_attn, b_attn, W_proj, b_proj)
    res = run_bass_kernel_spmd(nc, in_maps, list(range(NCORES)), trace=trace,
                               **spmd_kwargs)
    out = np.stack([res.results[b]["out"] for b in range(len(in_maps))],
                   axis=0)
    return out, res


def kernel(x, W_attn, b_attn, W_proj, b_proj):
    out, _ = run(x, W_attn, b_attn, W_proj, b_proj)
    return out
